# revision 21
# baseline (speedup 1.0000x reference)
"""Local-window sparse attention on 8 Trainium2 NeuronCores.

Reference op: per (batch, head) softmax attention with additive band mask
|i-j| <= 7, then output projection.  B=2, N=2048, C=768, H=12, Dh=64.

Sharding: data-parallel over (batch, sequence quarter) -> 8 shards of 512
query rows each.  The +-7 local window means each shard only needs a 7-row
halo of x, so every core computes its output rows fully locally (QKV proj,
banded attention, output proj) with no collectives.  Host side transposes
x / weights into the layouts the TensorEngine wants and bakes the band +
sequence-edge mask into a per-core additive mask input.  All matmuls run
as float32r (full PE rate at free-dim >= 256).

Softmax is computed without max-subtraction (scores are O(1) after the
1/sqrt(Dh) scale, folded into wq host-side) and normalization is deferred:
PV uses V augmented with a ones column, producing unnormalized outputs and
the softmax denominator in one PSUM tile; a selector matmul broadcasts the
per-(head, query) reciprocal across the 64 head channels.
"""

import numpy as np

import concourse.bacc as bacc
import concourse.tile as tile
from concourse import mybir
from concourse.bass_utils import run_bass_kernel_spmd

F32 = mybir.dt.float32
F32R = mybir.dt.float32r

B, N, C = 2, 2048, 768
H, Dh, W = 12, 64, 7
NCORES = 8
NQ = N * B // NCORES          # 512 query rows per core
HALO = 8                      # halo rows each side (>= W, even for alignment)
EXT = NQ + 2 * HALO           # 528 ext rows
QB = 256                      # query block (free dim of score matmuls)
NQB = NQ // QB                # 2
WIN = QB + 2 * HALO           # 272 key window per query block
CK = C // 128                 # 6 contraction chunks
CHUNKS = [(0, 128), (128, 128), (256, WIN - 256)]   # window j-chunks
KGROUPS = [(0, 264), (264, 264)]                     # kT free-dim groups (even sizes)
VGROUPS = [(0, 384), (384, 384)]                     # v o-dim groups
VT_SIZES = [128, 128, 128, 128, EXT - 512]           # v natural row tiles
NEG = -1.0e9


def build_program(reps: int = 1, dbg: bool = False):
    nc = bacc.Bacc("TRN2", target_bir_lowering=False, debug=False,
                   num_devices=NCORES)
    dbg_outs = {}
    if dbg:
        dbg_outs["qt"] = nc.declare_dram_parameter("qt_dbg", [C, NQ], F32,
                                                   isOutput=True)
        dbg_outs["kt"] = nc.declare_dram_parameter("kt_dbg", [C, EXT], F32,
                                                   isOutput=True)
        dbg_outs["v"] = nc.declare_dram_parameter("v_dbg", [EXT, H * 65], F32,
                                                  isOutput=True)
        dbg_outs["den"] = nc.declare_dram_parameter("den_dbg", [H, NQ], F32,
                                                    isOutput=True)
        dbg_outs["rec"] = nc.declare_dram_parameter("rec_dbg", [H, NQ], F32,
                                                    isOutput=True)
        dbg_outs["ht"] = nc.declare_dram_parameter("ht_dbg", [C, NQ], F32,
                                                   isOutput=True)
        dbg_outs["pr"] = nc.declare_dram_parameter("pr_dbg", [WIN, QB], F32,
                                                   isOutput=True)

    xt_in = nc.declare_dram_parameter("xt", [C, EXT], F32R, isOutput=False)
    wq_in = nc.declare_dram_parameter("wq", [C, C], F32R, isOutput=False)
    wk_in = nc.declare_dram_parameter("wk", [C, C], F32R, isOutput=False)
    wv_in = nc.declare_dram_parameter("wv", [C, C], F32R, isOutput=False)
    wp_in = nc.declare_dram_parameter("wp", [C, C], F32R, isOutput=False)
    mk_in = nc.declare_dram_parameter("maskt", [NQB, WIN, QB], F32,
                                      isOutput=False)
    ones_in = nc.declare_dram_parameter("ones", [128, H, 1], F32R,
                                        isOutput=False)
    sel_in = nc.declare_dram_parameter("sel", [H, C], F32R, isOutput=False)
    bias_in = nc.declare_dram_parameter("bias", [128, CK], F32, isOutput=False)
    yt_out = nc.declare_dram_parameter("yt", [C, NQ], F32, isOutput=True)


    Exp = mybir.ActivationFunctionType.Exp
    Mult = mybir.AluOpType.mult

    with tile.TileContext(nc) as tc:
        with (
            nc.allow_low_precision(reason="float32r matmul pipeline"),
            tc.tile_pool(name="persist", bufs=1) as pp,
            tc.tile_pool(name="tmp", bufs=4) as tp,
            tc.tile_pool(name="probs", bufs=8) as prp,
            tc.tile_pool(name="psA", bufs=2, space="PSUM") as psA,
            tc.tile_pool(name="psB", bufs=4, space="PSUM") as psB,
            tc.tile_pool(name="psC", bufs=2, space="PSUM") as psC,
            tc.tile_pool(name="dramp", bufs=1, space="DRAM") as dp,
        ):
            def body(_it):
                # ---- input DMA ----
                xt = pp.tile([128, CK, EXT], F32R, tag="xt")
                wq = pp.tile([128, CK, C], F32R, tag="wq")
                wk = pp.tile([128, CK, C], F32R, tag="wk")
                wv = pp.tile([128, CK, C], F32R, tag="wv")
                wp = pp.tile([128, CK, C], F32R, tag="wp")
                xt_r = xt_in.rearrange("(a p) n -> p a n", p=128)
                wq_r = wq_in.rearrange("(a p) n -> p a n", p=128)
                wk_r = wk_in.rearrange("(a p) n -> p a n", p=128)
                wv_r = wv_in.rearrange("(a p) n -> p a n", p=128)
                wp_r = wp_in.rearrange("(a p) n -> p a n", p=128)
                for ck in range(CK):
                    nc.sync.dma_start(out=xt[:, ck, :], in_=xt_r[:, ck, :])
                    nc.sync.dma_start(out=wq[:, ck, :], in_=wq_r[:, ck, :])
                for ck in range(CK):
                    nc.sync.dma_start(out=wk[:, ck, :], in_=wk_r[:, ck, :])
                    nc.sync.dma_start(out=wv[:, ck, :], in_=wv_r[:, ck, :])
                masks = []
                for qb in range(NQB):
                    row = []
                    for ci, (off, sz) in enumerate(CHUNKS):
                        m = pp.tile([sz, QB], F32, tag=f"mk{qb}{ci}")
                        nc.sync.dma_start(out=m[:], in_=mk_in[qb, off:off + sz, :])
                        row.append(m)
                    masks.append(row)
                sel = pp.tile([H, C], F32R, tag="sel")
                nc.sync.dma_start(out=sel[:], in_=sel_in[:])
                biast = pp.tile([128, CK], F32, tag="bias")
                nc.sync.dma_start(out=biast[:], in_=bias_in[:])
                for ck in range(CK):
                    nc.sync.dma_start(out=wp[:, ck, :], in_=wp_r[:, ck, :])

                # ---- QKV projections ----
                qt = pp.tile([128, CK, NQ], F32R, tag="qt")
                for ot in range(CK):
                    ps = psA.tile([128, 512], F32, tag="mmA")
                    for ck in range(CK):
                        nc.tensor.matmul(
                            ps[:],
                            wq[:, ck, ot * 128:(ot + 1) * 128],
                            xt[:, ck, HALO:HALO + NQ],
                            start=(ck == 0), stop=(ck == CK - 1),
                        )
                    nc.vector.tensor_copy(out=qt[:, ot, :], in_=ps[:])

                kt = pp.tile([128, CK, EXT], F32R, tag="kt")
                for ot in range(CK):
                    for g0, gs in KGROUPS:
                        ps = psA.tile([128, 512], F32, tag="mmA")
                        for ck in range(CK):
                            nc.tensor.matmul(
                                ps[:, :gs],
                                wk[:, ck, ot * 128:(ot + 1) * 128],
                                xt[:, ck, g0:g0 + gs],
                                start=(ck == 0), stop=(ck == CK - 1),
                            )
                        nc.vector.tensor_copy(out=kt[:, ot, g0:g0 + gs],
                                              in_=ps[:, :gs])

                vna = []
                for t, nt in enumerate(VT_SIZES):
                    vt = pp.tile([nt, H, 65], F32R, tag=f"vna{t}")
                    nc.sync.dma_start(out=vt[:, :, 64:65], in_=ones_in[:nt])
                    vna.append(vt)
                for t, nt in enumerate(VT_SIZES):
                    for gi, (g0, gs) in enumerate(VGROUPS):
                        ps = psA.tile([128, 512], F32, tag="mmA")
                        for ck in range(CK):
                            nc.tensor.matmul(
                                ps[:nt, :gs],
                                xt[:, ck, t * 128:t * 128 + nt],
                                wv[:, ck, g0:g0 + gs],
                                start=(ck == 0), stop=(ck == CK - 1),
                            )
                        nc.vector.tensor_copy(
                            out=vna[t][:, gi * 6:(gi + 1) * 6, 0:64],
                            in_=ps[:nt, :gs].rearrange("p (a b) -> p a b", b=64),
                        )

                # ---- banded attention ----
                # psv row 64 (softmax denominator) can only be copied at the
                # same partition base (engines cannot shift partitions), so
                # stage it on partition 64 and roundtrip through DRAM to get
                # the [H, NQ] head-per-partition layout.
                den64 = pp.tile([65, H, NQ], F32, tag="den64")
                den_dram = dp.tile([1, H, NQ], F32, tag="dend")
                ht = pp.tile([128, CK, NQ], F32R, tag="ht")
                for qb in range(NQB):
                    for h in range(H):
                        ti, pb = h // 2, (h % 2) * 64
                        probs = []
                        for ci, (off, sz) in enumerate(CHUNKS):
                            pss = psB.tile([128, QB], F32, tag="mmB")
                            nc.tensor.matmul(
                                pss[:sz, :],
                                kt[pb:pb + 64, ti, qb * QB + off:qb * QB + off + sz],
                                qt[pb:pb + 64, ti, qb * QB:(qb + 1) * QB],
                                start=True, stop=True,
                            )
                            ssb = tp.tile([128, QB], F32, tag="ssb")
                            nc.vector.tensor_add(out=ssb[:sz, :], in0=pss[:sz, :],
                                                 in1=masks[qb][ci][:sz, :])
                            pr = prp.tile([128, QB], F32R, tag="pr")
                            nc.scalar.activation(out=pr[:sz, :], in_=ssb[:sz, :],
                                                 func=Exp)
                            probs.append(pr)
                            if dbg and qb == 0 and h == 0:
                                nc.sync.dma_start(
                                    out=dbg_outs["pr"][off:off + sz, :],
                                    in_=pr[:sz, :].bitcast(F32))
                        psv = psC.tile([65, QB], F32, tag="pv")
                        for ci, (off, sz) in enumerate(CHUNKS):
                            nc.tensor.matmul(
                                psv[:],
                                vna[2 * qb + ci][:sz, h, :],
                                probs[ci][:sz, :],
                                start=(ci == 0), stop=(ci == len(CHUNKS) - 1),
                            )
                        nc.vector.tensor_copy(
                            out=ht[pb:pb + 64, ti, qb * QB:(qb + 1) * QB],
                            in_=psv[0:64, :],
                        )
                        nc.scalar.copy(
                            out=den64[64:65, h, qb * QB:(qb + 1) * QB],
                            in_=psv[64:65, :],
                        )

                # ---- normalize + output projection ----
                nc.sync.dma_start(out=den_dram[:], in_=den64[64:65, :, :])
                den = pp.tile([H, NQ], F32, tag="den")
                nc.sync.dma_start(out=den[:], in_=den_dram[0])
                rec = pp.tile([H, NQ], F32R, tag="rec")
                nc.vector.reciprocal(out=rec[:], in_=den[:])
                for ct in range(CK):
                    ps = psA.tile([128, 512], F32, tag="mmA")
                    nc.tensor.matmul(ps[:], sel[:, ct * 128:(ct + 1) * 128],
                                     rec[:], start=True, stop=True)
                    nc.vector.tensor_tensor(ht[:, ct, :], ht[:, ct, :], ps[:],
                                            Mult)
                if dbg:
                    qt_r = dbg_outs["qt"].rearrange("(a p) n -> p a n", p=128)
                    kt_r = dbg_outs["kt"].rearrange("(a p) n -> p a n", p=128)
                    ht_r = dbg_outs["ht"].rearrange("(a p) n -> p a n", p=128)
                    for ck in range(CK):
                        nc.sync.dma_start(out=qt_r[:, ck, :],
                                          in_=qt[:, ck, :].bitcast(F32))
                        nc.sync.dma_start(out=kt_r[:, ck, :],
                                          in_=kt[:, ck, :].bitcast(F32))
                        nc.sync.dma_start(out=ht_r[:, ck, :],
                                          in_=ht[:, ck, :].bitcast(F32))
                    for t, nt in enumerate(VT_SIZES):
                        nc.sync.dma_start(
                            out=dbg_outs["v"][t * 128:t * 128 + nt, :],
                            in_=vna[t][:, :, :].bitcast(F32))
                    nc.sync.dma_start(out=dbg_outs["den"][:], in_=den[:])
                    nc.sync.dma_start(out=dbg_outs["rec"][:],
                                      in_=rec[:].bitcast(F32))
                yt = pp.tile([128, CK, NQ], F32, tag="yt")
                for co in range(CK):
                    ps = psA.tile([128, 512], F32, tag="mmA")
                    for ck in range(CK):
                        nc.tensor.matmul(
                            ps[:],
                            wp[:, ck, co * 128:(co + 1) * 128],
                            ht[:, ck, :],
                            start=(ck == 0), stop=(ck == CK - 1),
                        )
                    nc.vector.tensor_scalar_add(out=yt[:, co, :], in0=ps[:],
                                                scalar1=biast[:, co:co + 1])
                yt_r = yt_out.rearrange("(a p) n -> p a n", p=128)
                for co in range(CK):
                    nc.sync.dma_start(out=yt_r[:, co, :], in_=yt[:, co, :])

            if reps == 1:
                body(0)
            else:
                with tc.For_i(0, reps, 1) as it:
                    body(it)

    nc.compile()
    return nc


def host_inputs(x, w_qkv, w_proj, b_proj):
    """Build the 8 per-core input maps from full inputs."""
    x = np.asarray(x, dtype=np.float32)
    w_qkv = np.asarray(w_qkv, dtype=np.float32)
    w_proj = np.asarray(w_proj, dtype=np.float32)
    b_proj = np.asarray(b_proj, dtype=np.float32)

    scale = Dh ** -0.5
    wq = np.ascontiguousarray((w_qkv[0:C] * scale).T)        # [C, C]
    wk = np.ascontiguousarray(w_qkv[C:2 * C].T)
    wv = np.ascontiguousarray(w_qkv[2 * C:3 * C].T)
    wp = np.ascontiguousarray(w_proj.T)
    ones = np.ones((128, H, 1), dtype=np.float32)
    sel = np.zeros((H, C), dtype=np.float32)
    for h in range(H):
        sel[h, h * Dh:(h + 1) * Dh] = 1.0
    bias = np.ascontiguousarray(b_proj.reshape(CK, 128).T)

    in_maps = []
    for c in range(NCORES):
        b = c // (NCORES // B)
        r0 = (c % (NCORES // B)) * NQ
        e0 = r0 - HALO
        xe = np.zeros((EXT, C), dtype=np.float32)
        lo, hi = max(0, e0), min(N, e0 + EXT)
        xe[lo - e0:hi - e0] = x[b, lo:hi]
        xt = np.ascontiguousarray(xe.T)                      # [C, EXT]

        maskt = np.full((NQB, WIN, QB), NEG, dtype=np.float32)
        wg = np.arange(WIN)[:, None]
        il = np.arange(QB)[None, :]
        for qb in range(NQB):
            jg = r0 + qb * QB - HALO + wg                    # global key row
            valid = (jg >= il + r0 + qb * QB - W) & (jg <= il + r0 + qb * QB + W) \
                & (jg >= 0) & (jg < N)
            maskt[qb][valid] = 0.0
        in_maps.append({
            "xt": xt, "wq": wq, "wk": wk, "wv": wv, "wp": wp,
            "maskt": maskt, "ones": ones, "sel": sel, "bias": bias,
        })
    return in_maps


_PROGRAM_CACHE = {}


def _get_program(reps: int = 1):
    if reps not in _PROGRAM_CACHE:
        _PROGRAM_CACHE[reps] = build_program(reps)
    return _PROGRAM_CACHE[reps]


def kernel(x, w_qkv, w_proj, b_proj):
    nc = _get_program(1)
    in_maps = host_inputs(x, w_qkv, w_proj, b_proj)
    res = run_bass_kernel_spmd(nc, in_maps, core_ids=list(range(NCORES)))
    y = np.empty((B, N, C), dtype=np.float32)
    for c in range(NCORES):
        b = c // (NCORES // B)
        r0 = (c % (NCORES // B)) * NQ
        y[b, r0:r0 + NQ] = res.results[c]["yt"].T
    return y
